# revision 1
# baseline (speedup 1.0000x reference)
"""CACombiner Trainium2 kernel: conv-projected efficient attention + FFN.

Data-parallel over batch: 8 batch elements -> 8 NeuronCores, identical SPMD
program per core. All heavy matmuls run as float32r (full PE rate); the
attention-weight path (exp(k), v, softmax(q), ctx) runs in bf16.
"""
import sys
sys.path.insert(0, "/opt/trn_rl_repo")
from contextlib import ExitStack

import numpy as np

import concourse.bass as bass
import concourse.tile as tile
from concourse import mybir, bacc
from concourse.bass_utils import run_bass_kernel_spmd
from concourse.alu_op_type import AluOpType

F32 = mybir.dt.float32
F32R = mybir.dt.float32r
BF16 = mybir.dt.bfloat16
AFT = mybir.ActivationFunctionType
Ax = mybir.AxisListType

B, C, L = 8, 512, 4096
H, DK = 8, 64
EPS = 1e-5
CC = C // 128          # 4 channel chunks
NL1 = L // 128         # 32 phase-1 l-tiles
NL2 = L // 512         # 8 phase-2 l-tiles

_CACHE = {}
LAST_RESULT = None


def _build_program():
    nc = bacc.Bacc("TRN2", target_bir_lowering=False, debug=False)

    def din(name, shape, dtype):
        return nc.dram_tensor(name, list(shape), dtype, kind="ExternalInput").ap()

    z1d = din("z1", (C, L), F32R)
    z2d = din("z2", (C, L), F32R)
    WqTt_d = din("WqTt", (128, CC, 512), F32R)
    bq_row_d = din("bq_row", (1, 512), F32R)
    WkvTt_d = din("WkvTt", (128, CC, 1024), F32R)
    WrTt_d = din("WrTt", (128, CC, 512), F32R)
    W1gTt_d = din("W1gTt", (128, CC, 1024), F32R)
    W2gTt_d = din("W2gTt", (128, 8, 512), F32R)
    U1W_d = din("U1W", (2, 1024), F32R)
    u2ct_d = din("u2ct", (128, 8), F32R)
    G2B_d = din("G2B", (2, 512), F32R)
    ivgt_d = din("ivgt", (128, CC), F32R)
    inv512_d = din("inv512", (128, 1), F32R)
    ones1x128_d = din("ones1x128", (1, 128), F32R)
    ident_d = din("ident", (128, 128), BF16)
    br_c_d = din("br_c", (128, CC), F32)
    bv_c_d = din("bv_c", (128, CC), F32)
    be2_c_d = din("be2_c", (128, CC), F32)
    eps_c_d = din("eps_c", (128, 1), F32)
    ones_row_d = din("ones_row", (1, 512), F32R)
    outd = nc.dram_tensor("out", [C, L], F32, kind="ExternalOutput").ap()

    z1r = z1d.rearrange("(cc p) l -> p cc l", p=128)
    z2r = z2d.rearrange("(cc p) l -> p cc l", p=128)

    mm = nc.tensor.matmul
    tt = nc.vector.tensor_tensor
    ts = nc.vector.tensor_scalar
    stt = nc.vector.scalar_tensor_tensor
    act = nc.scalar.activation

    with tile.TileContext(nc) as tc, ExitStack() as ctx:
        cpool = ctx.enter_context(tc.tile_pool(name="consts", bufs=1))

        def const_tile(shape, dtype, src, tag):
            t = cpool.tile(list(shape), dtype, tag=tag, name=tag)
            nc.sync.dma_start(t[:], src)
            return t

        WqTt = const_tile((128, CC, 512), F32R, WqTt_d, "WqTt")
        bq_row = const_tile((1, 512), F32R, bq_row_d, "bq_row")
        WkvTt = const_tile((128, CC, 1024), F32R, WkvTt_d, "WkvTt")
        WrTt = const_tile((128, CC, 512), F32R, WrTt_d, "WrTt")
        W1gTt = const_tile((128, CC, 1024), F32R, W1gTt_d, "W1gTt")
        W2gTt = const_tile((128, 8, 512), F32R, W2gTt_d, "W2gTt")
        U1W = const_tile((2, 1024), F32R, U1W_d, "U1W")
        u2ct = const_tile((128, 8), F32R, u2ct_d, "u2ct")
        G2B = const_tile((2, 512), F32R, G2B_d, "G2B")
        ivgt = const_tile((128, CC), F32R, ivgt_d, "ivgt")
        inv512 = const_tile((128, 1), F32R, inv512_d, "inv512")
        ones1x128 = const_tile((1, 128), F32R, ones1x128_d, "ones1x128")
        ident = const_tile((128, 128), BF16, ident_d, "ident")
        br_c = const_tile((128, CC), F32, br_c_d, "br_c")
        bv_c = const_tile((128, CC), F32, bv_c_d, "bv_c")
        be2_c = const_tile((128, CC), F32, be2_c_d, "be2_c")
        eps_c = const_tile((128, 1), F32, eps_c_d, "eps_c")
        ones_row = const_tile((1, 512), F32R, ones_row_d, "ones_row")

        # persistent across phases
        qsm = cpool.tile([128, CC, L], BF16, tag="qsm", name="qsm")      # softmaxed q, channels-first
        ctxbd = [cpool.tile([128, 128], BF16, tag=f"ctxbd{p}", name=f"ctxbd{p}") for p in range(CC)]

        # ---------------- Phase 1: q softmax + k/v + ctx accumulation ----------------
        with ExitStack() as p1:
            lp1 = p1.enter_context(tc.tile_pool(name="lp1", bufs=2))
            ps_ctx = p1.enter_context(tc.tile_pool(name="ps_ctx", bufs=1, space="PSUM"))
            ps_w = p1.enter_context(tc.tile_pool(name="ps_w", bufs=1, space="PSUM"))

            ctxps = [ps_ctx.tile([128, 129], F32, tag=f"ctx{p}", name=f"ctxps{p}") for p in range(CC)]

            for lt in range(NL1):
                sl = slice(lt * 128, (lt + 1) * 128)
                z1c = lp1.tile([128, CC, 128], F32R, tag="z1c")
                nc.sync.dma_start(z1c[:], z1r[:, :, sl])
                z2c = lp1.tile([128, CC, 128], F32R, tag="z2c")
                nc.sync.dma_start(z2c[:], z2r[:, :, sl])

                # qT [l,128][o,512] = z1^T Wq^T + bq
                qps = ps_w.tile([128, 512], F32, tag="qps")
                for cc in range(CC):
                    mm(qps[:], z1c[:, cc, :], WqTt[:, cc, :], start=(cc == 0), stop=False)
                mm(qps[:], ones1x128[:], bq_row[:], start=False, stop=True)

                # exp + per-head sums (ACT accumulate), then normalize
                EqT = lp1.tile([128, 512], F32, tag="EqT")
                Sq = lp1.tile([128, 8], F32, tag="Sq")
                for h in range(H):
                    hs = slice(h * 64, (h + 1) * 64)
                    act(EqT[:, hs], qps[:, hs], AFT.Exp, accum_out=Sq[:, h:h + 1])
                rq = lp1.tile([128, 8], F32, tag="rq")
                nc.vector.reciprocal(rq[:], Sq[:])
                qsmT = lp1.tile([128, 512], BF16, tag="qsmT")
                tt(qsmT[:].rearrange("p (g x) -> p g x", x=64),
                   EqT[:].rearrange("p (g x) -> p g x", x=64),
                   rq[:].unsqueeze(2).broadcast_to([128, 8, 64]), AluOpType.mult)

                # transpose qsmT back to channels-first into qsm
                tps = ps_w.tile([128, 512], BF16, tag="tps")
                for cc in range(CC):
                    cs = slice(cc * 128, (cc + 1) * 128)
                    nc.tensor.transpose(tps[:, cs], qsmT[:, cs], ident[:])
                nc.vector.tensor_copy(
                    qsm[:, :, sl],
                    tps[:].rearrange("p (cc x) -> p cc x", x=128))

                # kT | vT
                kvps = ps_w.tile([128, 1024], F32, tag="kvps")
                for cc in range(CC):
                    mm(kvps[:, 0:512], z2c[:, cc, :], WkvTt[:, cc, 0:512],
                       start=(cc == 0), stop=(cc == CC - 1))
                for cc in range(CC):
                    mm(kvps[:, 512:1024], z2c[:, cc, :], WkvTt[:, cc, 512:1024],
                       start=(cc == 0), stop=(cc == CC - 1))
                EkT = lp1.tile([128, 512], BF16, tag="EkT")
                act(EkT[:], kvps[:, 0:512], AFT.Exp)
                vT = lp1.tile([128, 516], BF16, tag="vT")
                nc.vector.tensor_copy(
                    vT[:].rearrange("p (pr x) -> p pr x", pr=4)[:, :, 0:128],
                    kvps[:, 512:1024].rearrange("p (pr x) -> p pr x", pr=4))
                nc.vector.memset(vT[:].rearrange("p (pr x) -> p pr x", pr=4)[:, :, 128:129], 1.0)

                # ctx accumulation: per head-pair [2heads-k, 2heads-v | S]
                for pr in range(CC):
                    mm(ctxps[pr][:], EkT[:, pr * 128:(pr + 1) * 128],
                       vT[:, pr * 129:(pr + 1) * 129],
                       start=(lt == 0), stop=(lt == NL1 - 1), skip_group_check=True)

            # finalize ctx: normalize rows by S, build block-diagonal bf16 tiles
            for pr in range(CC):
                rs = lp1.tile([128, 1], F32, tag="rs")
                nc.vector.reciprocal(rs[:], ctxps[pr][:, 128:129])
                nc.vector.memset(ctxbd[pr][:], 0.0)
                ts(ctxbd[pr][0:64, 0:64], ctxps[pr][0:64, 0:64], rs[0:64, :], None,
                   AluOpType.mult)
                ts(ctxbd[pr][64:128, 64:128], ctxps[pr][64:128, 64:128], rs[64:128, :], None,
                   AluOpType.mult)

        # ---------------- Phase 2: attention apply + reprojection + LN/FFN ----------------
        with ExitStack() as p2:
            lp2 = p2.enter_context(tc.tile_pool(name="lp2", bufs=2))
            lph = p2.enter_context(tc.tile_pool(name="lph", bufs=1))
            ps_big = p2.enter_context(tc.tile_pool(name="ps_big", bufs=5, space="PSUM"))
            ps_row = p2.enter_context(tc.tile_pool(name="ps_row", bufs=2, space="PSUM"))

            for lt in range(NL2):
                sl = slice(lt * 512, (lt + 1) * 512)
                z1res = lp2.tile([128, CC, 512], F32R, tag="z1res", bufs=1)
                nc.sync.dma_start(z1res[:], z1r[:, :, sl])

                # att[v,l] = ctx_bd @ qsm + bv
                att = []
                for pr in range(CC):
                    aps = ps_big.tile([128, 512], F32, tag="big")
                    mm(aps[:], ctxbd[pr][:], qsm[:, pr, sl], start=True, stop=True)
                    a = lph.tile([128, 512], F32R, tag=f"att{pr}")
                    ts(a[:], aps[:], bv_c[:, pr:pr + 1], None, AluOpType.add)
                    att.append(a)

                # z = Wr att + br + z1
                zt = []
                for ot in range(CC):
                    zps = ps_big.tile([128, 512], F32, tag="big")
                    for pr in range(CC):
                        mm(zps[:], WrTt[:, pr, ot * 128:(ot + 1) * 128], att[pr][:],
                           start=(pr == 0), stop=(pr == CC - 1))
                    z = lph.tile([128, 512], F32R, tag=f"z{ot}")
                    stt(z[:], zps[:], br_c[:, ot:ot + 1], z1res[:, ot, :].bitcast(F32),
                        AluOpType.add, AluOpType.add)
                    zt.append(z)

                # LN1 stats rows
                mups = ps_row.tile([1, 512], F32, tag="row")
                for ot in range(CC):
                    mm(mups[:], inv512[:], zt[ot][:], start=(ot == 0), stop=(ot == CC - 1))
                e2ps = ps_row.tile([1, 512], F32, tag="row")
                for ot in range(CC):
                    zsq = lp2.tile([128, 512], F32R, tag="zsq")
                    act(zsq[:], zt[ot][:].bitcast(F32), AFT.Square)
                    mm(e2ps[:], inv512[:], zsq[:], start=(ot == 0), stop=(ot == CC - 1))
                murow = lp2.tile([1, 512], F32, tag="murow", bufs=1)
                nc.vector.tensor_copy(murow[:], mups[:])
                musq = lp2.tile([1, 512], F32, tag="musq", bufs=1)
                tt(musq[:], murow[:], murow[:], AluOpType.mult)
                varrow = lp2.tile([1, 512], F32, tag="varrow", bufs=1)
                tt(varrow[:], e2ps[:], musq[:], AluOpType.subtract)
                sig = lp2.tile([1, 512], F32, tag="sig", bufs=1)
                act(sig[:], varrow[:], AFT.Sqrt, bias=eps_c[0:1, :])
                rhs2 = lp2.tile([2, 512], F32R, tag="rhs2", bufs=1)
                ts(rhs2[0:1, :], mups[:], -1.0, None, AluOpType.mult)
                sigR = lp2.tile([1, 512], F32R, tag="sigR", bufs=1)
                nc.vector.tensor_copy(sigR[:], sig[:])
                nc.sync.dma_start(rhs2[1:2, :], sigR[:])
                invsF = lp2.tile([1, 512], F32, tag="invsF", bufs=1)
                nc.vector.reciprocal(invsF[:], sig[:])
                invs = lp2.tile([1, 512], F32R, tag="invs", bufs=1)
                nc.vector.tensor_copy(invs[:], invsF[:])
                bc = ps_big.tile([128, 512], F32, tag="big")
                mm(bc[:], ones1x128[:], invs[:], start=True, stop=True)
                invsb = lp2.tile([128, 512], F32, tag="invsb", bufs=1)
                nc.vector.tensor_copy(invsb[:], bc[:])

                # FFN1 + ELU + FFN2 accumulation (j-outer so hE slots rotate)
                f2ps = [ps_big.tile([128, 512], F32, tag="big", name=f"f2ps{o2}")
                        for o2 in range(CC)]
                mu2 = ps_row.tile([1, 512], F32, tag="row", name="mu2")
                for j in range(8):
                    fps = ps_big.tile([128, 512], F32, tag="big", name="fps")
                    for cc in range(CC):
                        mm(fps[:], W1gTt[:, cc, j * 128:(j + 1) * 128], zt[cc][:],
                           start=(cc == 0), stop=False)
                    mm(fps[:], U1W[:, j * 128:(j + 1) * 128], rhs2[:], start=False, stop=True)
                    hp = lp2.tile([128, 512], F32, tag="hp")
                    tt(hp[:], fps[:], invsb[:], AluOpType.mult)
                    E = lp2.tile([128, 512], F32, tag="E")
                    act(E[:], hp[:], AFT.Exp)
                    nc.gpsimd.tensor_scalar(E[:], E[:], 1.0, -1.0, AluOpType.min,
                                            AluOpType.add)
                    he = lph.tile([128, 512], F32R, tag="hE", bufs=3, name="he")
                    stt(he[:], hp[:], 0.0, E[:], AluOpType.max, AluOpType.add)
                    for o2 in range(CC):
                        mm(f2ps[o2][:], W2gTt[:, j, o2 * 128:(o2 + 1) * 128], he[:],
                           start=(j == 0), stop=False, skip_group_check=True)
                    mm(mu2[:], u2ct[:, j:j + 1], he[:], start=(j == 0), stop=(j == 7),
                       skip_group_check=True)
                rhs2b = lp2.tile([2, 512], F32R, tag="rhs2b", bufs=1)
                nc.sync.dma_start(rhs2b[0:1, :], ones_row[:])
                negmu2 = lp2.tile([1, 512], F32R, tag="negmu2", bufs=1)
                ts(negmu2[:], mu2[:], -1.0, B2MEAN_PLACEHOLDER, AluOpType.mult,
                   AluOpType.subtract)
                nc.sync.dma_start(rhs2b[1:2, :], negmu2[:])
                yg = []
                for o2 in range(CC):
                    mm(f2ps[o2][:], G2B[:, o2 * 128:(o2 + 1) * 128], rhs2b[:],
                       start=False, stop=True, skip_group_check=True)
                    y = lph.tile([128, 512], F32, tag=f"yg{o2}", name=f"yg{o2}")
                    nc.vector.tensor_copy(y[:], f2ps[o2][:])
                    yg.append(y)

                # LN2 variance + apply
                v2ps = ps_row.tile([1, 512], F32, tag="row")
                for o2 in range(CC):
                    sq2 = lp2.tile([128, 512], F32R, tag="sq2")
                    act(sq2[:], yg[o2][:], AFT.Square)
                    mm(v2ps[:], ivgt[:, o2:o2 + 1], sq2[:], start=(o2 == 0),
                       stop=(o2 == CC - 1))
                sig2 = lp2.tile([1, 512], F32, tag="sig2", bufs=1)
                act(sig2[:], v2ps[:], AFT.Sqrt, bias=eps_c[0:1, :])
                invs2F = lp2.tile([1, 512], F32, tag="invs2F", bufs=1)
                nc.vector.reciprocal(invs2F[:], sig2[:])
                invs2 = lp2.tile([1, 512], F32R, tag="invs2", bufs=1)
                nc.vector.tensor_copy(invs2[:], invs2F[:])
                bc2 = ps_big.tile([128, 512], F32, tag="big")
                mm(bc2[:], ones1x128[:], invs2[:], start=True, stop=True)
                invsb2 = lp2.tile([128, 512], F32, tag="invsb2", bufs=1)
                nc.vector.tensor_copy(invsb2[:], bc2[:])
                for o2 in range(CC):
                    tt(yg[o2][:], yg[o2][:], invsb2[:], AluOpType.mult)
                    ot_t = lp2.tile([128, 512], F32, tag="ot")
                    nc.gpsimd.tensor_scalar(ot_t[:], yg[o2][:], be2_c[:, o2:o2 + 1],
                                            None, AluOpType.add)
                    nc.sync.dma_start(outd[o2 * 128:(o2 + 1) * 128, sl], ot_t[:])

    nc.compile()
    return nc


def _prep_consts(Wq, bq, Wk, bk, Wv, bv, Wr, br, g1, be1, W1, b1, W2, b2, g2, be2):
    f = np.float32
    WqT = np.ascontiguousarray(Wq.T, dtype=f)                       # [c, o]
    WkvT = np.concatenate([Wk.T, Wv.T], axis=1).astype(f)           # [c, k|v]
    WrT = np.ascontiguousarray(Wr.T, dtype=f)                       # [v, o]
    W1g = (W1 * g1[None, :]).astype(f)                              # [1024, c]
    W1gT = np.ascontiguousarray(W1g.T)                              # [c, 1024]
    W2g = (W2 * g2[:, None]).astype(f)                              # [c, 1024h]
    W2gT = np.ascontiguousarray(W2g.T)                              # [h, c]
    u1 = W1g.sum(axis=1).astype(f)
    w1bb = (W1 @ be1 + b1).astype(f)
    u2 = (W2.sum(axis=0) / 512.0).astype(f)
    ivg = (1.0 / (512.0 * g2 * g2)).astype(f)
    b2mean = float(np.mean(b2))

    def chunkT(a, n):          # [n*128, m] -> [128, n, m]
        return np.ascontiguousarray(a.reshape(n, 128, -1).transpose(1, 0, 2))

    def colsT(v, n):           # [n*128] -> [128, n]
        return np.ascontiguousarray(v.reshape(n, 128).T)

    return {
        "WqTt": chunkT(WqT, CC),
        "bq_row": bq.reshape(1, 512).astype(f),
        "WkvTt": chunkT(WkvT, CC),
        "WrTt": chunkT(WrT, CC),
        "W1gTt": chunkT(W1gT, CC),
        "W2gTt": chunkT(W2gT, 8),
        "U1W": np.stack([u1, w1bb]).astype(f),
        "u2ct": colsT(u2, 8),
        "G2B": np.stack([(g2 * b2).astype(f), g2.astype(f)]),
        "ivgt": colsT(ivg, CC),
        "inv512": np.full((128, 1), 1.0 / 512.0, dtype=f),
        "ones1x128": np.ones((1, 128), dtype=f),
        "ident": np.eye(128, dtype=f).astype(np.dtype("bfloat16") if False else f),
        "br_c": colsT(br.astype(f), CC),
        "bv_c": colsT(bv.astype(f), CC),
        "be2_c": colsT(be2.astype(f), CC),
        "eps_c": np.full((128, 1), EPS, dtype=f),
        "ones_row": np.ones((1, 512), dtype=f),
    }, b2mean


def kernel(**inputs):
    global LAST_RESULT
    import ml_dtypes
    z1 = np.asarray(inputs["z1"], dtype=np.float32)
    z2 = np.asarray(inputs["z2"], dtype=np.float32)
    consts, b2mean = _prep_consts(
        *[np.asarray(inputs[k], dtype=np.float32) for k in
          ["Wq", "bq", "Wk", "bk", "Wv", "bv", "Wr", "br", "g1", "be1",
           "W1", "b1", "W2", "b2", "g2", "be2"]])
    consts["ident"] = np.eye(128, dtype=ml_dtypes.bfloat16)

    key = ("prog", round(b2mean * 1e9))
    if key not in _CACHE:
        global B2MEAN_PLACEHOLDER
        B2MEAN_PLACEHOLDER = b2mean
        _CACHE.clear()
        _CACHE[key] = _build_program()
    nc = _CACHE[key]

    in_maps = []
    for b in range(B):
        m = dict(consts)
        m["z1"] = np.ascontiguousarray(z1[b])
        m["z2"] = np.ascontiguousarray(z2[b])
        in_maps.append(m)

    import os
    trace = bool(int(os.environ.get("KERNEL_TRACE", "0")))
    res = run_bass_kernel_spmd(nc, in_maps, list(range(B)), trace=trace)
    LAST_RESULT = res
    out = np.stack([res.results[b]["out"] for b in range(B)], axis=0)
    return out.astype(np.float32)


B2MEAN_PLACEHOLDER = 0.0



# revision 3
# speedup vs baseline: 1.9547x; 1.9547x over previous
"""CACombiner Trainium2 kernel: conv-projected efficient attention + FFN.

Data-parallel over batch: 8 batch elements -> 8 NeuronCores, identical SPMD
program per core. Attention path (q/k/v projections, reprojection) runs as
fp8e4m3 DoubleRow matmuls (K=256 per instruction); ctx/apply in bf16; the
FFN runs in bf16 (fp8 does not meet the accuracy budget there).

Structure per core:
  phase 1  (16 x 256-l pairs): q softmax -> qsm, exp(k), v, ctx/S accumulation
  phase 2a (8 x 512-l tiles):  att apply, reprojection + residual, LN1 -> zr
  phase 2b (8 x 512-l tiles):  FFN1 + ELU + FFN2, LN2 -> out
"""
import sys
sys.path.insert(0, "/opt/trn_rl_repo")
from contextlib import ExitStack

import numpy as np

import concourse.bass as bass
import concourse.tile as tile
from concourse import mybir, bacc
from concourse.bass_utils import run_bass_kernel_spmd
from concourse.alu_op_type import AluOpType

F32 = mybir.dt.float32
F32R = mybir.dt.float32r
BF16 = mybir.dt.bfloat16
F8 = mybir.dt.float8e4
AFT = mybir.ActivationFunctionType
Ax = mybir.AxisListType
DR = mybir.MatmulPerfMode.DoubleRow

B, C, L = 8, 512, 4096
H, DK = 8, 64
EPS = 1e-5
CC = C // 128           # 4 channel chunks
NP1 = L // 256          # 16 phase-1 pair-tiles (2x128 l)
NL2 = L // 512          # 8 phase-2 l-tiles

SW = 32.0               # weight scale for fp8
SA = 256.0              # att scale for fp8
ZDESC = 1.0 / (SW * SA)  # descale for reprojection output

_CACHE = {}
LAST_RESULT = None


def _build_program(gates):
    (HAS_BQ, HAS_BK, HAS_BV, HAS_BR, HAS_B1, HAS_G2, HAS_B2, HAS_BE2) = gates
    nc = bacc.Bacc("TRN2", target_bir_lowering=False, debug=False)

    def din(name, shape, dtype):
        return nc.dram_tensor(name, list(shape), dtype, kind="ExternalInput").ap()

    z1d = din("z1", (C, L), F32)
    z2d = din("z2", (C, L), F32)
    Wq8_d = din("Wq8", (128, CC, 512), F8)
    Wkv8_d = din("Wkv8", (128, CC, 1024), F8)
    Wr8_d = din("Wr8", (128, CC, 512), F8)
    W1gb_d = din("W1gb", (128, CC, 1024), BF16)
    W2gb_d = din("W2gb", (128, 8, 512), BF16)
    U1neg_d = din("U1neg", (2, 1024), BF16)
    inv512b_d = din("inv512b", (128, 1), BF16)
    ones1x128_d = din("ones1x128", (1, 128), F32R)
    identb_d = din("identb", (128, 128), BF16)
    ones_f8_d = din("ones_f8", (128, 2, 1), F8)
    eps11_d = din("eps11", (1, 1), F32)
    # gated bias constants (all-zero in the common case)
    bq32_d = din("bq32", (1, 512), F32R)
    bk32_d = din("bk32", (1, 512), F32R)
    bv_c_d = din("bv_c", (128, CC), F32)
    br_c_d = din("br_c", (128, CC), F32)
    b2_c_d = din("b2_c", (128, CC), F32)
    g2_c_d = din("g2_c", (128, CC), F32)
    be2_c_d = din("be2_c", (128, CC), F32)
    outd = nc.dram_tensor("out", [C, L], F32, kind="ExternalOutput").ap()

    z1r = z1d.rearrange("(cc p) l -> p cc l", p=128)
    z2r = z2d.rearrange("(cc p) l -> p cc l", p=128)
    outr = outd.rearrange("(cc p) l -> p cc l", p=128)

    mm = nc.tensor.matmul
    tt = nc.vector.tensor_tensor
    ts = nc.vector.tensor_scalar
    stt = nc.vector.scalar_tensor_tensor
    act = nc.scalar.activation
    pts = nc.gpsimd.tensor_scalar
    pstt = nc.gpsimd.scalar_tensor_tensor
    ptt = nc.gpsimd.tensor_tensor
    pcopy = nc.gpsimd.tensor_copy

    with tile.TileContext(nc) as tc, ExitStack() as ctx:
        cpool = ctx.enter_context(tc.tile_pool(name="consts", bufs=1))

        def const_tile(shape, dtype, src, tag):
            t = cpool.tile(list(shape), dtype, tag=tag, name=tag)
            nc.sync.dma_start(t[:], src)
            return t

        Wq8 = const_tile((128, CC, 512), F8, Wq8_d, "Wq8")
        Wkv8 = const_tile((128, CC, 1024), F8, Wkv8_d, "Wkv8")
        Wr8 = const_tile((128, CC, 512), F8, Wr8_d, "Wr8")
        W1gb = const_tile((128, CC, 1024), BF16, W1gb_d, "W1gb")
        W2gb = const_tile((128, 8, 512), BF16, W2gb_d, "W2gb")
        U1neg = const_tile((2, 1024), BF16, U1neg_d, "U1neg")
        inv512b = const_tile((128, 1), BF16, inv512b_d, "inv512b")
        ones1x128 = const_tile((1, 128), F32R, ones1x128_d, "ones1x128")
        identb = const_tile((128, 128), BF16, identb_d, "identb")
        ones_f8 = const_tile((128, 2, 1), F8, ones_f8_d, "ones_f8")
        eps11 = const_tile((1, 1), F32, eps11_d, "eps11")
        if HAS_BQ:
            bq32 = const_tile((1, 512), F32R, bq32_d, "bq32")
        if HAS_BK:
            bk32 = const_tile((1, 512), F32R, bk32_d, "bk32")
        if HAS_BV:
            bv_c = const_tile((128, CC), F32, bv_c_d, "bv_c")
        if HAS_BR:
            br_c = const_tile((128, CC), F32, br_c_d, "br_c")
        if HAS_B2:
            b2_c = const_tile((128, CC), F32, b2_c_d, "b2_c")
        if HAS_G2:
            g2_c = const_tile((128, CC), F32, g2_c_d, "g2_c")
        if HAS_BE2:
            be2_c = const_tile((128, CC), F32, be2_c_d, "be2_c")

        # persistent across phases
        qsm = cpool.tile([128, CC, L], BF16, tag="qsm", name="qsm")
        ctxbd = [cpool.tile([128, 128], BF16, tag=f"ctxbd{p}", name=f"ctxbd{p}")
                 for p in range(CC)]
        rs = cpool.tile([128, CC], F32, tag="rs", name="rs")
        zr_all = cpool.tile([128, NL2, CC, 512], BF16, tag="zr", name="zr_all")
        nfold = 2 if HAS_B1 else 1
        mur_all = cpool.tile([2, NL2, 512], BF16, tag="mur", name="mur_all")

        # ---------------- Phase 1: q softmax + k/v + ctx accumulation --------
        with ExitStack() as p1:
            lp = p1.enter_context(tc.tile_pool(name="lp1", bufs=2))
            psw = p1.enter_context(tc.tile_pool(name="psw", bufs=2, space="PSUM"))
            pst = p1.enter_context(tc.tile_pool(name="pst", bufs=1, space="PSUM"))
            psc = p1.enter_context(tc.tile_pool(name="psc", bufs=1, space="PSUM"))

            ctxps = psc.tile([128, CC, 128], F32, tag="ctxps", name="ctxps")
            Sps = psc.tile([128, CC], F32, tag="Sps", name="Sps")

            for p in range(NP1):
                l0 = p * 256
                sl = slice(l0, l0 + 256)
                z1c = lp.tile([128, CC, 256], F32, tag="z1c")
                nc.sync.dma_start(z1c[:], z1r[:, :, sl])
                z2c = lp.tile([128, CC, 256], F32, tag="z2c")
                nc.sync.dma_start(z2c[:], z2r[:, :, sl])
                z1f8 = lp.tile([128, CC, 256], F8, tag="z1f8")
                pcopy(z1f8[:], z1c[:])
                z2f8 = lp.tile([128, CC, 256], F8, tag="z2f8")
                pcopy(z2f8[:], z2c[:])

                # qT [l,o] fp8 DoubleRow (values = SW * q_true)
                qps = psw.tile([128, 2, 512], F32, tag="pw", name="qps")
                for i in range(2):
                    ls = slice(i * 128, (i + 1) * 128)
                    mm(qps[:, i, :], z1f8[:, 0:2, ls], Wq8[:, 0:2, :],
                       start=True, stop=False, perf_mode=DR)
                    mm(qps[:, i, :], z1f8[:, 2:4, ls], Wq8[:, 2:4, :],
                       start=False, stop=not HAS_BQ, perf_mode=DR)
                    if HAS_BQ:
                        mm(qps[:, i, :], ones1x128[:], bq32[:],
                           start=False, stop=True)
                EqT = lp.tile([128, 2, 512], BF16, tag="EqT")
                act(EqT[:], qps[:], AFT.Exp, scale=1.0 / SW)
                Sq = lp.tile([128, 2, 8], F32, tag="Sq")
                nc.vector.tensor_reduce(
                    Sq[:], EqT[:].rearrange("p i (h x) -> p i h x", x=64),
                    Ax.X, AluOpType.add)
                rq = lp.tile([128, 2, 8], F32, tag="rq")
                nc.vector.reciprocal(rq[:], Sq[:])
                qsmT = lp.tile([128, 2, 512], BF16, tag="qsmT")
                tt(qsmT[:].rearrange("p i (h x) -> p i h x", x=64),
                   EqT[:].rearrange("p i (h x) -> p i h x", x=64),
                   rq[:].unsqueeze(3).broadcast_to([128, 2, 8, 64]),
                   AluOpType.mult)

                # k fp8 DoubleRow (values = SW * k_true)
                kps = psw.tile([128, 2, 512], F32, tag="pw", name="kps")
                for i in range(2):
                    ls = slice(i * 128, (i + 1) * 128)
                    mm(kps[:, i, :], z2f8[:, 0:2, ls], Wkv8[:, 0:2, 0:512],
                       start=True, stop=False, perf_mode=DR)
                    mm(kps[:, i, :], z2f8[:, 2:4, ls], Wkv8[:, 2:4, 0:512],
                       start=False, stop=not HAS_BK, perf_mode=DR)
                    if HAS_BK:
                        mm(kps[:, i, :], ones1x128[:], bk32[:],
                           start=False, stop=True)
                EkT = lp.tile([128, 2, 512], F8, tag="EkT")
                act(EkT[:], kps[:], AFT.Exp, scale=1.0 / SW)

                # v fp8 DoubleRow
                vps = psw.tile([128, 2, 512], F32, tag="pw", name="vps")
                for i in range(2):
                    ls = slice(i * 128, (i + 1) * 128)
                    mm(vps[:, i, :], z2f8[:, 0:2, ls], Wkv8[:, 0:2, 512:1024],
                       start=True, stop=False, perf_mode=DR)
                    mm(vps[:, i, :], z2f8[:, 2:4, ls], Wkv8[:, 2:4, 512:1024],
                       start=False, stop=True, perf_mode=DR)
                vT = lp.tile([128, 2, 512], F8, tag="vT")
                if HAS_BV:
                    for cc in range(CC):
                        cs = slice(cc * 128, (cc + 1) * 128)
                        ts(vT[:, :, cs], vps[:, :, cs], 1.0 / SW,
                           bv_c[:, cc:cc + 1], AluOpType.mult, AluOpType.add)
                else:
                    ts(vT[:], vps[:], 1.0 / SW, None, AluOpType.mult)

                # ctx/S accumulation over l
                for pr in range(CC):
                    ks = slice(pr * 128, (pr + 1) * 128)
                    mm(ctxps[:, pr, :], EkT[:, :, ks], vT[:, :, ks],
                       start=(p == 0), stop=(p == NP1 - 1), perf_mode=DR,
                       skip_group_check=True)
                    mm(Sps[:, pr:pr + 1], EkT[:, :, ks], ones_f8[:],
                       start=(p == 0), stop=(p == NP1 - 1), perf_mode=DR,
                       skip_group_check=True)

                # transpose qsmT -> channels-first qsm (consumed in phase 2a)
                tps = pst.tile([128, 2, 512], BF16, tag="tps")
                for i in range(2):
                    for cc in range(CC):
                        cs = slice(cc * 128, (cc + 1) * 128)
                        nc.tensor.transpose(tps[:, i, cs], qsmT[:, i, cs],
                                            identb[:])
                act(qsm[:, :, sl].rearrange("p cc (i x) -> p i cc x", x=128),
                    tps[:].rearrange("p i (cc x) -> p i cc x", x=128), AFT.Copy)

            # finalize: ctx_bd = (ctx / S) * SA (+bv), block-diagonal bf16
            nc.vector.reciprocal(rs[:], Sps[:])
            for pr in range(CC):
                nc.vector.memset(ctxbd[pr][:], 0.0)
                for h2 in range(2):
                    s = slice(h2 * 64, (h2 + 1) * 64)
                    ts(ctxbd[pr][s, s], ctxps[s, pr, s], rs[s, pr:pr + 1], SA,
                       AluOpType.mult, AluOpType.mult)

        # ------------- Phase 2a: apply + reprojection + LN1 -> zr ------------
        with ExitStack() as p2a:
            lpa = p2a.enter_context(tc.tile_pool(name="lpa", bufs=2))
            psb = p2a.enter_context(tc.tile_pool(name="psb", bufs=2, space="PSUM"))
            psr = p2a.enter_context(tc.tile_pool(name="psr", bufs=3, space="PSUM"))

            for t in range(NL2):
                sl = slice(t * 512, (t + 1) * 512)
                z1res = lpa.tile([128, CC, 512], F32, tag="z1res")
                nc.sync.dma_start(z1res[:], z1r[:, :, sl])

                # att = ctx_bd @ qsm (values = SA * att_true), in halves
                att8 = lpa.tile([128, CC, 512], F8, tag="att8")
                z = lpa.tile([128, CC, 512], BF16, tag="z")
                for half in range(2):
                    aps = psb.tile([128, 2, 512], F32, tag="big", name="aps")
                    for i in range(2):
                        pr = half * 2 + i
                        mm(aps[:, i, :], ctxbd[pr][:], qsm[:, pr, sl],
                           start=True, stop=True)
                    hs = slice(half * 2, half * 2 + 2)
                    if half == 0:
                        pts(att8[:, hs, :], aps[:], 1.0, None, AluOpType.mult)
                    else:
                        act(att8[:, hs, :], aps[:], AFT.Copy)

                # z = Wr att / (SW*SA) + z1  -> bf16, in halves
                for half in range(2):
                    zps = psb.tile([128, 2, 512], F32, tag="big", name="zps")
                    for i in range(2):
                        ot = half * 2 + i
                        os_ = slice(ot * 128, (ot + 1) * 128)
                        mm(zps[:, i, :], Wr8[:, 0:2, os_], att8[:, 0:2, :],
                           start=True, stop=False, perf_mode=DR)
                        mm(zps[:, i, :], Wr8[:, 2:4, os_], att8[:, 2:4, :],
                           start=False, stop=True, perf_mode=DR)
                    hs = slice(half * 2, half * 2 + 2)
                    if half == 0:
                        stt(z[:, hs, :], zps[:], ZDESC, z1res[:, hs, :],
                            AluOpType.mult, AluOpType.add)
                    else:
                        pstt(z[:, hs, :], zps[:], ZDESC, z1res[:, hs, :],
                             AluOpType.mult, AluOpType.add)
                    if HAS_BR:
                        for i in range(2):
                            cc = half * 2 + i
                            ts(z[:, cc, :], z[:, cc, :], br_c[:, cc:cc + 1],
                               None, AluOpType.add)
                zsq = lpa.tile([128, CC, 512], BF16, tag="zsq")
                tt(zsq[:], z[:], z[:], AluOpType.mult)

                # LN1 stats
                mups = psr.tile([1, 512], F32, tag="row", name="mups")
                for cc in range(CC):
                    mm(mups[:], inv512b[:], z[:, cc, :], start=(cc == 0),
                       stop=(cc == CC - 1))
                e2ps = psr.tile([1, 512], F32, tag="row", name="e2ps")
                for cc in range(CC):
                    mm(e2ps[:], inv512b[:], zsq[:, cc, :], start=(cc == 0),
                       stop=(cc == CC - 1))
                musq = lpa.tile([1, 512], F32, tag="musq", bufs=1)
                tt(musq[:], mups[:], mups[:], AluOpType.mult)
                varrow = lpa.tile([1, 512], F32, tag="varrow", bufs=1)
                ptt(varrow[:], e2ps[:], musq[:], AluOpType.subtract)
                sig = lpa.tile([1, 512], F32, tag="sig", bufs=1)
                act(sig[:], varrow[:], AFT.Sqrt, bias=eps11[0:1, :])
                rrow = lpa.tile([1, 512], F32, tag="rrow", bufs=1)
                nc.vector.reciprocal(rrow[:], sig[:])
                ptt(mur_all[0:1, t, :], mups[:], rrow[:], AluOpType.mult)
                if HAS_B1:
                    nc.vector.memset(mur_all[1:2, t, :], 1.0)
                rbc = psr.tile([128, 512], F32, tag="row", name="rbc")
                mm(rbc[:], ones1x128[:], rrow[:].bitcast(F32R), start=True,
                   stop=True)
                tt(zr_all[:, t, :, :], z[:],
                   rbc[:].unsqueeze(1).broadcast_to([128, CC, 512]),
                   AluOpType.mult)

        # ------------- Phase 2b: FFN1 + ELU + FFN2 + LN2 -> out --------------
        with ExitStack() as p2b:
            lpb = p2b.enter_context(tc.tile_pool(name="lpb", bufs=2))
            lph = p2b.enter_context(tc.tile_pool(name="lph", bufs=1))
            psF = p2b.enter_context(tc.tile_pool(name="psF", bufs=1, space="PSUM"))
            psf = p2b.enter_context(tc.tile_pool(name="psf", bufs=3, space="PSUM"))

            for t in range(NL2):
                sl = slice(t * 512, (t + 1) * 512)
                f2ps = psF.tile([128, CC, 512], F32, tag="f2", name="f2ps")
                for j in range(8):
                    js = slice(j * 128, (j + 1) * 128)
                    fps = psf.tile([128, 512], F32, tag="fps", name="fps")
                    for cc in range(CC):
                        mm(fps[:], W1gb[:, cc, js], zr_all[:, t, cc, :],
                           start=(cc == 0), stop=False)
                    mm(fps[:], U1neg[0:nfold, js], mur_all[0:nfold, t, :],
                       start=False, stop=True)
                    E = lpb.tile([128, 512], BF16, tag="E")
                    act(E[:], fps[:], AFT.Exp)
                    E2 = lpb.tile([128, 512], BF16, tag="E2")
                    pts(E2[:], E[:], 1.0, -1.0, AluOpType.min, AluOpType.add)
                    he = lph.tile([128, 512], BF16, tag="he", bufs=3, name="he")
                    tt(he[:], fps[:], E2[:], AluOpType.max)
                    for o2 in range(CC):
                        mm(f2ps[:, o2, :], W2gb[:, j, o2 * 128:(o2 + 1) * 128],
                           he[:], start=(j == 0), stop=(j == 7),
                           skip_group_check=True)

                # LN2 (stats-based)
                y = lpb.tile([128, CC, 512], BF16, tag="y")
                if HAS_B2:
                    for cc in range(CC):
                        pts(y[:, cc, :], f2ps[:, cc, :], b2_c[:, cc:cc + 1],
                            None, AluOpType.add)
                else:
                    pcopy(y[:, 0:2, :], f2ps[:, 0:2, :])
                    act(y[:, 2:4, :], f2ps[:, 2:4, :], AFT.Copy)
                ysq = lpb.tile([128, CC, 512], BF16, tag="ysq")
                tt(ysq[:], y[:], y[:], AluOpType.mult)
                muy = psf.tile([1, 512], F32, tag="fps", name="muy")
                for cc in range(CC):
                    mm(muy[:], inv512b[:], y[:, cc, :], start=(cc == 0),
                       stop=(cc == CC - 1))
                e2y = psf.tile([1, 512], F32, tag="fps", name="e2y")
                for cc in range(CC):
                    mm(e2y[:], inv512b[:], ysq[:, cc, :], start=(cc == 0),
                       stop=(cc == CC - 1))
                mur2 = lpb.tile([1, 512], F32, tag="mur2", bufs=1)
                nc.vector.tensor_copy(mur2[:], muy[:])
                musq2 = lpb.tile([1, 512], F32, tag="musq2", bufs=1)
                tt(musq2[:], mur2[:], mur2[:], AluOpType.mult)
                var2 = lpb.tile([1, 512], F32, tag="var2", bufs=1)
                ptt(var2[:], e2y[:], musq2[:], AluOpType.subtract)
                sig2 = lpb.tile([1, 512], F32, tag="sig2", bufs=1)
                act(sig2[:], var2[:], AFT.Sqrt, bias=eps11[0:1, :])
                r2row = lpb.tile([1, 512], F32, tag="r2row", bufs=1)
                nc.vector.reciprocal(r2row[:], sig2[:])
                r2bc = psf.tile([128, 512], F32, tag="fps", name="r2bc")
                mm(r2bc[:], ones1x128[:], r2row[:].bitcast(F32R), start=True,
                   stop=True)
                mu2bc = psf.tile([128, 512], F32, tag="fps", name="mu2bc")
                mm(mu2bc[:], ones1x128[:], mur2[:].bitcast(F32R), start=True,
                   stop=True)
                yc = lpb.tile([128, CC, 512], BF16, tag="yc")
                tt(yc[:], y[:],
                   mu2bc[:].unsqueeze(1).broadcast_to([128, CC, 512]),
                   AluOpType.subtract)
                outt = lpb.tile([128, CC, 512], F32, tag="outt")
                tt(outt[:], yc[:],
                   r2bc[:].unsqueeze(1).broadcast_to([128, CC, 512]),
                   AluOpType.mult)
                if HAS_G2:
                    for cc in range(CC):
                        ts(outt[:, cc, :], outt[:, cc, :], g2_c[:, cc:cc + 1],
                           None, AluOpType.mult)
                if HAS_BE2:
                    for cc in range(CC):
                        ts(outt[:, cc, :], outt[:, cc, :], be2_c[:, cc:cc + 1],
                           None, AluOpType.add)
                nc.sync.dma_start(outr[:, :, sl], outt[:])

    nc.compile()
    return nc


def _prep_consts(Wq, bq, Wk, bk, Wv, bv, Wr, br, g1, be1, W1, b1, W2, b2, g2, be2):
    import ml_dtypes
    f = np.float32
    f8 = ml_dtypes.float8_e4m3
    bf = ml_dtypes.bfloat16

    def chunkT(a, n):          # [n*128, m] -> [128, n, m]
        return np.ascontiguousarray(a.reshape(n, 128, -1).transpose(1, 0, 2))

    def colsT(v, n):           # [n*128] -> [128, n]
        return np.ascontiguousarray(v.reshape(n, 128).T)

    WqT = np.ascontiguousarray(Wq.T, dtype=f)                       # [c, o]
    WkvT = np.concatenate([Wk.T, Wv.T], axis=1).astype(f)           # [c, k|v]
    WrT = np.ascontiguousarray(Wr.T, dtype=f)                       # [v, o]
    g2_is_one = bool(np.all(g2 == 1.0))
    W1g = (W1 * g1[None, :]).astype(f)                              # [1024, c]
    W2u = (W2 * g2[:, None]).astype(f) if g2_is_one else W2.astype(f)
    u1 = W1g.sum(axis=1).astype(f)
    w1bb = (W1 @ be1 + b1).astype(f)

    gates = (
        bool(np.any(bq != 0)), bool(np.any(bk != 0)), bool(np.any(bv != 0)),
        bool(np.any(br != 0)), bool(np.any(w1bb != 0)),
        not g2_is_one, bool(np.any(b2 != 0)), bool(np.any(be2 != 0)),
    )

    consts = {
        "Wq8": chunkT(WqT * SW, CC).astype(f8),
        "Wkv8": chunkT(WkvT * SW, CC).astype(f8),
        "Wr8": chunkT(WrT * SW, CC).astype(f8),
        "W1gb": chunkT(np.ascontiguousarray(W1g.T), CC).astype(bf),
        "W2gb": chunkT(np.ascontiguousarray(W2u.T), 8).astype(bf),
        "U1neg": np.stack([-u1, w1bb]).astype(bf),
        "inv512b": np.full((128, 1), 1.0 / 512.0, dtype=bf),
        "ones1x128": np.ones((1, 128), dtype=f),
        "identb": np.eye(128, dtype=bf),
        "ones_f8": np.ones((128, 2, 1), dtype=f8),
        "eps11": np.full((1, 1), EPS, dtype=f),
        "bq32": (bq * SW).reshape(1, 512).astype(f),
        "bk32": (bk * SW).reshape(1, 512).astype(f),
        "bv_c": colsT(bv.astype(f), CC),
        "br_c": colsT(br.astype(f), CC),
        "b2_c": colsT(b2.astype(f), CC),
        "g2_c": colsT(g2.astype(f), CC),
        "be2_c": colsT(be2.astype(f), CC),
    }
    return consts, gates


def kernel(**inputs):
    global LAST_RESULT
    z1 = np.asarray(inputs["z1"], dtype=np.float32)
    z2 = np.asarray(inputs["z2"], dtype=np.float32)
    consts, gates = _prep_consts(
        *[np.asarray(inputs[k], dtype=np.float32) for k in
          ["Wq", "bq", "Wk", "bk", "Wv", "bv", "Wr", "br", "g1", "be1",
           "W1", "b1", "W2", "b2", "g2", "be2"]])

    key = ("prog", gates)
    if key not in _CACHE:
        _CACHE.clear()
        _CACHE[key] = _build_program(gates)
    nc = _CACHE[key]

    in_maps = []
    for b in range(B):
        m = dict(consts)
        m["z1"] = np.ascontiguousarray(z1[b])
        m["z2"] = np.ascontiguousarray(z2[b])
        in_maps.append(m)

    import os
    trace = bool(int(os.environ.get("KERNEL_TRACE", "0")))
    res = run_bass_kernel_spmd(nc, in_maps, list(range(B)), trace=trace)
    LAST_RESULT = res
    out = np.stack([res.results[b]["out"] for b in range(B)], axis=0)
    return out.astype(np.float32)
